# revision 1
# baseline (speedup 1.0000x reference)
"""Trainium2 Bass kernel for nn_BlockAttentionResidual (sparse block attention + BitNet-style quantized MLP).

Sharding: sequence-block data parallelism. The block attention is independent per
512-token block, so each of the 8 cores owns 1024 contiguous tokens (2 blocks) of
one batch element and runs the ENTIRE layer on them with zero collectives.
  core c -> batch c//4, tokens [(c%4)*1024, (c%4+1)*1024)

Numerics (validated against the reference in fp64/numpy emulation, l2rel ~2.2e-3):
 - bitlinear matmuls (qkv/o_proj/ffn_down) run on integer-valued bf16 operands
   (act-quant ints in [-128,127] and ternary weights are exact in bf16; PSUM
   accumulates fp32) -> exactly reproduces the reference quantized matmul.
 - attention (scores/softmax/ctx) in bf16 with fp32 accumulation; softmax skips
   max-subtraction (max|score| ~ 2.9, mathematically identical).
 - ffn_up (the only unquantized matmul) runs in float32r (~tf32) at bf16 speed.
"""

import os
import numpy as np
import ml_dtypes

import concourse.bass as bass
import concourse.mybir as mybir
import concourse.tile as tile
from concourse import bacc
from concourse.bass_utils import run_bass_kernel_spmd
from concourse.masks import make_identity

F32 = mybir.dt.float32
F32R = mybir.dt.float32r
BF16 = mybir.dt.bfloat16
F16 = mybir.dt.float16
AX = mybir.AxisListType
OP = mybir.AluOpType
ACTF = mybir.ActivationFunctionType

# model dims
H = 2048
NH = 16
HD = 128
NB = 8
INTER = 4096        # 2*H
EPS = 1e-5
THETA = 10000.0
B, S = 2, 4096
BT = 512            # tokens per attention block
NCORES = 8
R = 1024            # tokens per core
NT = R // 128       # 8 token tiles per core
MAGIC = np.float32(1.5 * 2 ** 23)   # fp32 round-to-nearest-even magic
SCALE_QK = float(HD ** -0.5)


def _quantize_weight(nc, tc, pool, psum_pool, w_dram, n_ktiles, ncols, out_writer,
                     singles, tag, col_chunk=None):
    """Ternarize a [n_ktiles*128, ncols] f32 weight.  Two sweeps over w_dram
    (in [128, col_chunk] tiles): sweep1 accumulates |w| sums (ACT), sweep2 emits
    ternary bf16 tiles via out_writer((kk, chunk), tern_tile).
    Returns dqw = mean|w| as a [128,1] broadcast sbuf tile."""
    if col_chunk is None:
        col_chunk = ncols
    nch = ncols // col_chunk
    asum = singles.tile([128, n_ktiles * nch], F32, tag=f"asum_{tag}")
    for kk in range(n_ktiles):
        for h in range(nch):
            wt = pool.tile([128, col_chunk], F32, tag="wq_in")
            nc.sync.dma_start(wt[:], w_dram[kk * 128:(kk + 1) * 128,
                                            h * col_chunk:(h + 1) * col_chunk])
            junk = pool.tile([128, col_chunk], BF16, tag="wq_g1")
            nc.scalar.activation(junk[:], wt[:], ACTF.Abs,
                                 accum_out=asum[:, kk * nch + h:kk * nch + h + 1])
    tot = singles.tile([128, 1], F32, tag=f"tot_{tag}")
    nc.vector.tensor_reduce(tot[:], asum[:], AX.X, OP.add)
    ones_col = singles.tile([128, 1], F32, tag=f"ones_{tag}")
    nc.vector.memset(ones_col[:], 1.0)
    ps = psum_pool.tile([1, 1], F32, tag=f"wqp_{tag}")
    nc.tensor.matmul(ps[:], tot[:], ones_col[:], start=True, stop=True)
    mean1 = singles.tile([1, 1], F32, tag=f"mean_{tag}")
    nc.vector.tensor_scalar(mean1[:], ps[:], 1.0 / (n_ktiles * 128 * ncols), 1e-5,
                            OP.mult, OP.max)
    s1 = singles.tile([1, 1], F32, tag=f"s1_{tag}")
    nc.vector.reciprocal(s1[:], mean1[:])
    ones_row = singles.tile([1, 128], F32, tag=f"onesr_{tag}")
    nc.vector.memset(ones_row[:], 1.0)
    psb = psum_pool.tile([128, 2], F32, tag=f"wqb_{tag}")
    nc.tensor.matmul(psb[:, 0:1], ones_row[:], s1[:], start=True, stop=True)
    nc.tensor.matmul(psb[:, 1:2], ones_row[:], mean1[:], start=True, stop=True)
    s_bc = singles.tile([128, 1], F32, tag=f"sbc_{tag}")
    nc.vector.tensor_copy(s_bc[:], psb[:, 0:1])
    dqw = singles.tile([128, 1], F32, tag=f"dqw_{tag}")
    nc.vector.tensor_copy(dqw[:], psb[:, 1:2])
    half_p = singles.tile([128, 1], F32, tag=f"halfp_{tag}")
    nc.vector.memset(half_p[:], 0.5)
    half_n = singles.tile([128, 1], F32, tag=f"halfn_{tag}")
    nc.vector.memset(half_n[:], -0.5)
    # ternary * 2: sign(w*s - 0.5) + sign(w*s + 0.5) in {-2, 0, 2};
    # the 0.5 factor is folded into dqw by the caller.
    for kk in range(n_ktiles):
        for h in range(nch):
            wt = pool.tile([128, col_chunk], F32, tag="wq_in")
            nc.sync.dma_start(wt[:], w_dram[kk * 128:(kk + 1) * 128,
                                            h * col_chunk:(h + 1) * col_chunk])
            g1 = pool.tile([128, col_chunk], BF16, tag="wq_g1")
            g2 = pool.tile([128, col_chunk], BF16, tag="wq_g2")
            nc.scalar.activation(g1[:], wt[:], ACTF.Sign, bias=half_n[:], scale=s_bc[:])
            nc.scalar.activation(g2[:], wt[:], ACTF.Sign, bias=half_p[:], scale=s_bc[:])
            tern = pool.tile([128, col_chunk], BF16, tag="wq_t")
            nc.vector.tensor_tensor(tern[:], g1[:], g2[:], OP.add)
            out_writer((kk, h), tern)
    return dqw


def _act_quant(nc, pool, src_ap, ncols, s_store, dq_store, scratch_tag, out_bf, magic_ap, amax_in=None):
    """Per-row int8-grid quantization of src_ap [128, ncols] -> out_bf (bf16 ints).
    Writes 127/amax into s_store [128,1] slice, amax/127 into dq_store."""
    if amax_in is None:
        amax = pool.tile([128, 1], F32, tag=f"amax_{scratch_tag}")
        nc.vector.tensor_reduce(amax[:], src_ap, AX.X, OP.max, apply_absolute_value=True)
        amax_in = amax
    amc = pool.tile([128, 1], F32, tag=f"amc_{scratch_tag}")
    nc.vector.tensor_scalar_max(amc[:], amax_in[:], 1e-5)
    rec = pool.tile([128, 1], F32, tag=f"rec_{scratch_tag}")
    nc.vector.reciprocal(rec[:], amc[:])
    nc.vector.tensor_scalar_mul(s_store, rec[:], 127.0)
    nc.vector.tensor_scalar_mul(dq_store, amc[:], 1.0 / 127.0)
    # round(src*s): ACT does src*s + MAGIC (f32), DVE subtracts MAGIC -> bf16 ints
    mg = pool.tile([128, ncols], F32, tag=f"mg_{scratch_tag}")
    nc.scalar.activation(mg[:], src_ap, ACTF.Identity, bias=magic_ap, scale=s_store)
    nc.vector.tensor_scalar_sub(out_bf, mg[:], float(MAGIC))


def build_program():
    nc = bacc.Bacc(None, target_bir_lowering=False)

    # ---- I/O ----
    x_in = nc.declare_dram_parameter("x_sh", [R, H], F32, isOutput=False)
    cos_in = nc.declare_dram_parameter("cos_sh", [NT, 128, NH * 64], F32, isOutput=False)
    sin_in = nc.declare_dram_parameter("sin_sh", [NT, 128, NH * 64], F32, isOutput=False)
    anw_in = nc.declare_dram_parameter("attn_norm_w", [H], F32, isOutput=False)
    fnw_in = nc.declare_dram_parameter("ffn_norm_w", [H], F32, isOutput=False)
    wqkv_in = nc.declare_dram_parameter("wqkv_t", [H, 3 * H], F32, isOutput=False)
    wo_in = nc.declare_dram_parameter("wo_t", [H, H], F32, isOutput=False)
    wup_in = nc.declare_dram_parameter("wup_t", [H, 2 * INTER], F32, isOutput=False)
    wdn_in = nc.declare_dram_parameter("wdn_t", [INTER, H], F32, isOutput=False)
    out_d = nc.declare_dram_parameter("out_sh", [R, H], F32, isOutput=True)

    # ---- internal DRAM scratch ----
    wqkv_q = nc.dram_tensor("wqkv_q", [48, 16, 128, 128], BF16)
    wo_q = nc.dram_tensor("wo_q", [16, 16, 128, 128], BF16)
    wdn_q = nc.dram_tensor("wdn_q", [16, 32, 128, 128], BF16)
    x1_d = nc.dram_tensor("x1_d", [NT, 128, H], F32)
    act_d = nc.dram_tensor("act_d", [NT, 128, INTER], F16)

    with tile.TileContext(nc) as tc:
        perm = tc.alloc_tile_pool(name="perm", bufs=1)
        ident = perm.tile([128, 128], F32)
        make_identity(nc, ident)
        magic_t = perm.tile([128, 1], F32)
        nc.vector.memset(magic_t[:], float(MAGIC))
        magic_ap = magic_t[:]
        s_all = perm.tile([128, NT], F32)
        dqa_all = perm.tile([128, NT], F32)
        s_c = perm.tile([128, NT], F32)
        dqc_all = perm.tile([128, NT], F32)
        s_a = perm.tile([128, NT], F32)
        dqact_all = perm.tile([128, NT], F32)
        dqw_qkv_p = perm.tile([128, 1], F32)
        dqw_o_p = perm.tile([128, 1], F32)
        dqw_dn_p = perm.tile([128, 1], F32)

        # ------------ weight quantization (DRAM roundtrip) ------------
        with tc.tile_pool(name="wqp", bufs=2) as wpool, \
             tc.tile_pool(name="wqs", bufs=1) as wsingles, \
             tc.tile_pool(name="wq_psum", bufs=1, space="PSUM") as psum_s:
            def qkv_writer(cc, tern):
                # tern [128, 3072] covering featcols 24*cc..24*cc+24 of k-tile kk
                kk, half = cc
                nc.sync.dma_start(
                    wqkv_q[24 * half:24 * (half + 1), kk, :, :].rearrange("c p f -> p c f"),
                    tern[:])
            dqw = _quantize_weight(nc, tc, wpool, psum_s, wqkv_in, 16, 3 * H,
                                   qkv_writer, wsingles, "qkv", col_chunk=3072)
            nc.vector.tensor_scalar_mul(dqw_qkv_p[:], dqw[:], 0.5)


        # slot1: xqT (N1..QKV) then ctx_sb (ATTN..CTXQ)
        slot1_pool = tc.alloc_tile_pool(name="slot1_pool", bufs=1)

        # ------------ attn rmsnorm + act-quant + transpose ------------
        xqT = slot1_pool.tile([128, 16, NT, 128], BF16, tag="slot1")
        with tc.tile_pool(name="npool", bufs=2) as npool, \
             tc.tile_pool(name="nwpool", bufs=1) as nwpool:
            anw_b = nwpool.tile([128, H], F32, tag="normw")
            ap0 = anw_in[:]
            nc.gpsimd.dma_start(out=anw_b[:], in_=bass.AP(
                tensor=ap0.tensor, offset=ap0.offset, ap=[[0, 128]] + list(ap0.ap)))
            for t in range(NT):
                xt = npool.tile([128, H], F32, tag="xt")
                nc.sync.dma_start(xt[:], x_in[t * 128:(t + 1) * 128, :])
                ssq = npool.tile([128, 1], F32, tag="ssq")
                junk = npool.tile([128, H], BF16, tag="njunk")
                nc.scalar.activation(junk[:], xt[:], ACTF.Square, accum_out=ssq[:])
                msq = npool.tile([128, 1], F32, tag="msq")
                nc.vector.tensor_scalar(msq[:], ssq[:], 1.0 / H, EPS, OP.mult, OP.add)
                sd = npool.tile([128, 1], F32, tag="sd")
                nc.scalar.activation(sd[:], msq[:], ACTF.Sqrt)
                rstd = npool.tile([128, 1], F32, tag="rstd")
                nc.vector.reciprocal(rstd[:], sd[:])
                h_t = npool.tile([128, H], F32, tag="h_t")
                nc.vector.tensor_scalar_mul(h_t[:], xt[:], rstd[:])
                nc.vector.tensor_tensor(h_t[:], h_t[:], anw_b[:], OP.mult)
                xq = npool.tile([128, H], BF16, tag="xq")
                _act_quant(nc, npool, h_t[:], H, s_all[:, t:t + 1], dqa_all[:, t:t + 1],
                           "n1", xq[:], magic_ap)
                nc.sync.dma_start_transpose(xqT[:, :, t, :], xq[:])

        # ------------ qkv matmul (token-major) + rope/dequant + transpose ------------
        arena = tc.alloc_tile_pool(name="arena", bufs=1)
        qT = arena.tile([128, NH, NT, 128], BF16, tag="qTslot")
        kT = arena.tile([128, NH, NT, 128], BF16, tag="kTslot")
        v_sb = arena.tile([128, NT, NH, 128], BF16, tag="vslot")
        with tc.tile_pool(name="qkpool", bufs=2) as qkpool, \
             tc.tile_pool(name="cs_pool", bufs=2) as cs_pool, \
             tc.tile_pool(name="cs1_pool", bufs=1) as cs1_pool, \
             tc.tile_pool(name="qkv_psum", bufs=4, space="PSUM") as psum_mm:
            for nn in range(12):
                wst = qkpool.tile([128, 16, 4, 128], BF16, tag="w_qkv")
                for c in range(4):
                    nc.sync.dma_start(wst[:, :, c, :], wqkv_q[4 * nn + c, :, :, :]
                                      .rearrange("k p f -> p k f"))
                is_v = nn >= 8
                if not is_v:
                    hsel0 = nn % 4
                    sl = slice(4 * hsel0 * 64, (4 * hsel0 + 4) * 64)
                    cos_st = cs1_pool.tile([128, NT, 4, 64], F32, tag="cos_st")
                    nc.sync.dma_start(cos_st[:], cos_in[:, :, sl]
                                      .rearrange("t p (c f) -> p t c f", c=4))
                    sin_st = cs1_pool.tile([128, NT, 4, 64], F32, tag="sin_st")
                    nc.sync.dma_start(sin_st[:], sin_in[:, :, sl]
                                      .rearrange("t p (c f) -> p t c f", c=4))
                for t in range(NT):
                    ps = psum_mm.tile([128, 512], F32, tag="ps_qkv")
                    for kk in range(16):
                        nc.tensor.matmul(ps[:], xqT[:, kk, t, :], wst[:, kk, :, :],
                                         start=(kk == 0), stop=(kk == 15))
                    dq_t = cs_pool.tile([128, 1], F32, tag="dq_t")
                    nc.vector.tensor_scalar(dq_t[:], dqa_all[:, t:t + 1], dqw_qkv_p[:],
                                            None, OP.mult)
                    if is_v:
                        hsel = nn - 8
                        nc.vector.tensor_scalar_mul(
                            v_sb[:, t, 4 * hsel:4 * hsel + 4, :],
                            ps[:].rearrange("p (c f) -> p c f", c=4), dq_t[:])
                    else:
                        dst = qT if nn < 4 else kT
                        hsel = nn % 4
                        qsc = cs_pool.tile([128, 4, 128], F32, tag="qsc")
                        nc.vector.tensor_scalar_mul(
                            qsc[:], ps[:].rearrange("p (c f) -> p c f", c=4), dq_t[:])
                        p1, p2 = qsc[:, :, 0:64], qsc[:, :, 64:128]
                        cosd, sind = cos_st[:, t, :, :], sin_st[:, t, :, :]
                        t1 = cs_pool.tile([128, 4, 64], F32, tag="rt1")
                        t2 = cs_pool.tile([128, 4, 64], F32, tag="rt2")
                        rot = cs_pool.tile([128, 4, 128], BF16, tag="rot")
                        nc.vector.tensor_tensor(t1[:], p1, cosd, OP.mult)
                        nc.gpsimd.tensor_tensor(t2[:], p2, sind, OP.mult)
                        nc.vector.tensor_tensor(rot[:, :, 0:64], t1[:], t2[:], OP.subtract)
                        nc.vector.tensor_tensor(t1[:], p2, cosd, OP.mult)
                        nc.gpsimd.tensor_tensor(t2[:], p1, sind, OP.mult)
                        nc.vector.tensor_tensor(rot[:, :, 64:128], t1[:], t2[:], OP.add)
                        nc.sync.dma_start_transpose(
                            dst[:, 4 * hsel:4 * hsel + 4, t, :],
                            rot[:].rearrange("p c f -> p (c f)"))

        # ------------ block attention ------------
        ctx_sb = slot1_pool.tile([128, NT, NH, 128], BF16, tag="slot1")
        with tc.tile_pool(name="apool", bufs=2) as apool, \
             tc.tile_pool(name="at_psum", bufs=4, space="PSUM") as psum_at, \
             tc.tile_pool(name="ctx_psum", bufs=2, space="PSUM") as psum_cx:
            for blk in range(2):
                for h in range(NH):
                    expT = [None] * 4
                    vaug = [None] * 4
                    for kt in range(4):
                        qn = 512 - kt * 128
                        pss = psum_at.tile([128, 512], F32, tag="ps_sc")
                        nc.tensor.matmul(
                            pss[:, 0:qn],
                            kT[:, h, blk * 4 + kt, :],
                            qT[:, h, blk * 4:(blk + 1) * 4, :]
                            .rearrange("p c f -> p (c f)")[:, kt * 128:512],
                            start=True, stop=True)
                        ex = apool.tile([128, 512], BF16, tag=f"expT{kt}")
                        nc.scalar.activation(ex[:, 0:qn], pss[:, 0:qn], ACTF.Exp,
                                             scale=SCALE_QK)
                        nc.gpsimd.affine_select(
                            out=ex[:, 0:128], in_=ex[:, 0:128],
                            compare_op=OP.is_ge, fill=0.0,
                            base=0, pattern=[[1, 128]], channel_multiplier=-1)
                        expT[kt] = ex
                        va = apool.tile([128, 132], BF16, tag=f"vaug{kt}")
                        nc.vector.tensor_copy(va[:, 0:128], v_sb[:, blk * 4 + kt, h, :])
                        nc.vector.memset(va[:, 128:129], 1.0)
                        vaug[kt] = va
                    for qt in range(4):
                        psc = psum_cx.tile([128, 132], F32, tag="ps_ctx")
                        for kt in range(qt + 1):
                            nc.tensor.matmul(psc[:, 0:129],
                                             expT[kt][:, (qt - kt) * 128:(qt - kt) * 128 + 128],
                                             vaug[kt][:, 0:129],
                                             start=(kt == 0), stop=(kt == qt))
                        rl = apool.tile([128, 1], F32, tag="rl")
                        nc.vector.reciprocal(rl[:], psc[:, 128:129])
                        nc.vector.tensor_scalar_mul(ctx_sb[:, blk * 4 + qt, h, :],
                                                    psc[:, 0:128], rl[:])

        # ------------ o_proj weight quant (overlaps attention) ------------
        with tc.tile_pool(name="wqp_o", bufs=2) as wpool_o, \
             tc.tile_pool(name="wqs_o", bufs=1) as wsingles_o, \
             tc.tile_pool(name="wq_psum_o", bufs=1, space="PSUM") as psum_so:
            def o_writer(cc, tern):
                kk = cc[0]
                nc.sync.dma_start(wo_q[:, kk, :, :].rearrange("c p f -> p c f"), tern[:])
            dqw = _quantize_weight(nc, tc, wpool_o, psum_so, wo_in, 16, H,
                                   o_writer, wsingles_o, "o")
            nc.vector.tensor_scalar_mul(dqw_o_p[:], dqw[:], 0.5)

        # ------------ ctx quant + transpose (ctxqT reuses qT slot) ------------
        ctxqT = arena.tile([128, 16, NT, 128], BF16, tag="qTslot")
        with tc.tile_pool(name="cqpool", bufs=2) as cqpool:
            for t in range(NT):
                cq = cqpool.tile([128, H], BF16, tag="cq")
                _act_quant(nc, cqpool, ctx_sb[:, t, :, :].rearrange("p c f -> p (c f)"),
                           H, s_c[:, t:t + 1], dqc_all[:, t:t + 1], "cq", cq[:], magic_ap)
                nc.sync.dma_start_transpose(ctxqT[:, :, t, :], cq[:])

        # ------------ o matmul + residual -> x1_d ------------
        with tc.tile_pool(name="opool", bufs=3) as opool, \
             tc.tile_pool(name="owpool", bufs=2) as owpool, \
             tc.tile_pool(name="o_psum", bufs=2, space="PSUM") as psum_o:
            for nn in range(4):
                wst = owpool.tile([128, 16, 4, 128], BF16, tag="wo_st")
                for c in range(4):
                    nc.sync.dma_start(wst[:, :, c, :], wo_q[4 * nn + c, :, :, :]
                                      .rearrange("k p f -> p k f"))
                for t in range(NT):
                    ps = psum_o.tile([128, 512], F32, tag="ps_o")
                    for kk in range(16):
                        nc.tensor.matmul(ps[:], ctxqT[:, kk, t, :], wst[:, kk, :, :],
                                         start=(kk == 0), stop=(kk == 15))
                    dq_t = opool.tile([128, 1], F32, tag="dq_ot")
                    nc.vector.tensor_scalar(dq_t[:], dqc_all[:, t:t + 1], dqw_o_p[:],
                                            None, OP.mult)
                    xs = opool.tile([128, 512], F32, tag="xs")
                    nc.sync.dma_start(xs[:], x_in[t * 128:(t + 1) * 128,
                                                  nn * 512:(nn + 1) * 512])
                    tmp = opool.tile([128, 512], F32, tag="o_tmp")
                    nc.vector.tensor_scalar_mul(tmp[:], ps[:], dq_t[:])
                    x1s = opool.tile([128, 512], F32, tag="x1s")
                    nc.vector.tensor_tensor(x1s[:], tmp[:], xs[:], OP.add)
                    nc.sync.dma_start(x1_d[t, :, nn * 512:(nn + 1) * 512], x1s[:])
        arena.release()
        slot1_pool.release()

        # hnT (N2..UP) then actqT (ACTQ..DOWN) share one 64KB slot
        arena2 = tc.alloc_tile_pool(name="arena2", bufs=1)
        hnT = arena2.tile([128, 16, NT, 128], F32R, tag="bigslot")

        # ------------ ffn norm + hn transpose (x1 reloaded) ------------
        with tc.tile_pool(name="n2pool", bufs=2) as n2pool, \
             tc.tile_pool(name="n2wpool", bufs=1) as n2wpool, \
             tc.tile_pool(name="tr2_psum", bufs=2, space="PSUM") as psum_tr:
            fnw_b = n2wpool.tile([128, H], F32, tag="normw2")
            ap0 = fnw_in[:]
            nc.gpsimd.dma_start(out=fnw_b[:], in_=bass.AP(
                tensor=ap0.tensor, offset=ap0.offset, ap=[[0, 128]] + list(ap0.ap)))
            for t in range(NT):
                x1_t = n2pool.tile([128, H], F32, tag="x1n")
                nc.sync.dma_start(x1_t[:], x1_d[t, :, :])
                ssq = n2pool.tile([128, 1], F32, tag="ssq2")
                junk = n2pool.tile([128, H], BF16, tag="njunk2")
                nc.scalar.activation(junk[:], x1_t[:], ACTF.Square, accum_out=ssq[:])
                msq = n2pool.tile([128, 1], F32, tag="msq2")
                nc.vector.tensor_scalar(msq[:], ssq[:], 1.0 / H, EPS, OP.mult, OP.add)
                sd = n2pool.tile([128, 1], F32, tag="sd2")
                nc.scalar.activation(sd[:], msq[:], ACTF.Sqrt)
                rstd = n2pool.tile([128, 1], F32, tag="rstd2")
                nc.vector.reciprocal(rstd[:], sd[:])
                hn_t = n2pool.tile([128, H], F32, tag="hn_t")
                nc.vector.tensor_scalar_mul(hn_t[:], x1_t[:], rstd[:])
                nc.vector.tensor_tensor(hn_t[:], hn_t[:], fnw_b[:], OP.mult)
                for kk in range(16):
                    pst = psum_tr.tile([128, 128], F32, tag="ps_tr")
                    nc.tensor.transpose(pst[:], hn_t[:, kk * 128:(kk + 1) * 128], ident[:])
                    nc.scalar.copy(hnT[:, kk, t, :], pst[:])

        # ------------ ffn up (fp32r) + silu*val -> act spill ------------
        with tc.tile_pool(name="upool", bufs=2) as upool, \
             tc.tile_pool(name="upool1", bufs=1) as upool1, \
             tc.tile_pool(name="fpool", bufs=3) as fpool, \
             tc.tile_pool(name="up_psum", bufs=2, space="PSUM") as psum_up, \
             tc.tile_pool(name="upv_psum", bufs=2, space="PSUM") as psum_upv:
            for i in range(8):   # paired gate/val strips of 512
                wgr = upool.tile([128, 16, 512], F32R, tag="wgr")
                nc.gpsimd.dma_start(wgr[:], wup_in[:, i * 512:(i + 1) * 512]
                                    .rearrange("(kk p) f -> p kk f", p=128))
                wvr = upool1.tile([128, 16, 512], F32R, tag="wvr")
                nc.gpsimd.dma_start(wvr[:], wup_in[:, INTER + i * 512:INTER + (i + 1) * 512]
                                    .rearrange("(kk p) f -> p kk f", p=128))
                for t in range(NT):
                    psg = psum_up.tile([128, 512], F32, tag="ps_g")
                    for kk in range(16):
                        nc.tensor.matmul(psg[:], hnT[:, kk, t, :], wgr[:, kk, :],
                                         start=(kk == 0), stop=(kk == 15))
                    sgm = fpool.tile([128, 512], F32, tag="sgm")
                    nc.scalar.activation(sgm[:], psg[:], ACTF.Sigmoid)
                    sg = fpool.tile([128, 512], F32, tag="sg")
                    nc.vector.tensor_tensor(sg[:], sgm[:], psg[:], OP.mult)
                    psv = psum_upv.tile([128, 512], F32, tag="ps_v")
                    for kk in range(16):
                        nc.tensor.matmul(psv[:], hnT[:, kk, t, :], wvr[:, kk, :],
                                         start=(kk == 0), stop=(kk == 15))
                    av = fpool.tile([128, 512], F16, tag="av")
                    nc.vector.tensor_tensor(av[:], sg[:], psv[:], OP.mult)
                    nc.sync.dma_start(act_d[t, :, i * 512:(i + 1) * 512], av[:])

        # ------------ ffn_down weight quant (overlaps ffn_up) ------------
        with tc.tile_pool(name="wqp_dn", bufs=2) as wpool_dn, \
             tc.tile_pool(name="wqs_dn", bufs=1) as wsingles_dn, \
             tc.tile_pool(name="wq_psum_dn", bufs=1, space="PSUM") as psum_sdn:
            def dn_writer(cc, tern):
                kk = cc[0]
                nc.sync.dma_start(wdn_q[:, kk, :, :].rearrange("c p f -> p c f"), tern[:])
            dqw = _quantize_weight(nc, tc, wpool_dn, psum_sdn, wdn_in, 32, H,
                                   dn_writer, wsingles_dn, "dn")
            nc.vector.tensor_scalar_mul(dqw_dn_p[:], dqw[:], 0.5)

        # ------------ act quant + transpose (actqT reuses hnT slot) ------------
        actqT = arena2.tile([128, 32, NT, 128], BF16, tag="bigslot")
        with tc.tile_pool(name="aqpool", bufs=2) as aqpool:
            for t in range(NT):
                at = aqpool.tile([128, INTER], F16, tag="at")
                for c in range(4):
                    nc.sync.dma_start(at[:, c * 1024:(c + 1) * 1024],
                                      act_d[t, :, c * 1024:(c + 1) * 1024])
                aq = aqpool.tile([128, INTER], BF16, tag="aq")
                _act_quant(nc, aqpool, at[:], INTER, s_a[:, t:t + 1],
                           dqact_all[:, t:t + 1], "aq", aq[:], magic_ap)
                nc.sync.dma_start_transpose(actqT[:, 0:16, t, :], aq[:, 0:2048])
                nc.sync.dma_start_transpose(actqT[:, 16:32, t, :], aq[:, 2048:4096])

        # ------------ ffn down + residual -> out ------------
        with tc.tile_pool(name="dpool", bufs=2) as dpool, \
             tc.tile_pool(name="dopool", bufs=3) as dopool, \
             tc.tile_pool(name="dn_psum", bufs=2, space="PSUM") as psum_dn:
            for nn in range(4):
                wst = dpool.tile([128, 32, 4, 128], BF16, tag="w_dn")
                for c in range(4):
                    nc.sync.dma_start(wst[:, :, c, :], wdn_q[4 * nn + c, :, :, :]
                                      .rearrange("k p f -> p k f"))
                for t in range(NT):
                    ps = psum_dn.tile([128, 512], F32, tag="ps_dn")
                    for kk in range(32):
                        nc.tensor.matmul(ps[:], actqT[:, kk, t, :], wst[:, kk, :, :],
                                         start=(kk == 0), stop=(kk == 31))
                    dq_t = dopool.tile([128, 1], F32, tag="dq_dt")
                    nc.vector.tensor_scalar(dq_t[:], dqact_all[:, t:t + 1], dqw_dn_p[:],
                                            None, OP.mult)
                    x1_t = dopool.tile([128, 512], F32, tag="x1_re")
                    nc.sync.dma_start(x1_t[:], x1_d[t, :, nn * 512:(nn + 1) * 512])
                    tmp = dopool.tile([128, 512], F32, tag="d_tmp")
                    nc.vector.tensor_scalar_mul(tmp[:], ps[:], dq_t[:])
                    ot = dopool.tile([128, 512], F32, tag="ot")
                    nc.vector.tensor_tensor(ot[:], tmp[:], x1_t[:], OP.add)
                    nc.sync.dma_start(out_d[t * 128:(t + 1) * 128,
                                            nn * 512:(nn + 1) * 512], ot[:])
        arena2.release()
        perm.release()

    nc.compile()
    return nc


_NC_CACHE = None


def _get_nc():
    global _NC_CACHE
    if _NC_CACHE is None:
        _NC_CACHE = build_program()
    return _NC_CACHE


def _host_inputs(x, attn_norm_w, ffn_norm_w, qkv_w, o_w, ffn_up_w, ffn_down_w):
    x = np.ascontiguousarray(np.asarray(x, np.float32))
    wqkv_t = np.ascontiguousarray(np.asarray(qkv_w, np.float32).T)
    wo_t = np.ascontiguousarray(np.asarray(o_w, np.float32).T)
    wup_t = np.ascontiguousarray(np.asarray(ffn_up_w, np.float32).T)
    wdn_t = np.ascontiguousarray(np.asarray(ffn_down_w, np.float32).T)
    anw = np.ascontiguousarray(np.asarray(attn_norm_w, np.float32))
    fnw = np.ascontiguousarray(np.asarray(ffn_norm_w, np.float32))

    inv = 1.0 / (THETA ** (np.arange(0, HD, 2, dtype=np.float32) / HD))
    tpos = np.arange(S, dtype=np.float32)
    fr = np.outer(tpos, inv)                     # [S, 64]
    cos = np.cos(fr).astype(np.float32)
    sin = np.sin(fr).astype(np.float32)
    # token-major per-core shards, replicated per head: [NT, 128, NH*64]
    cos_rep = np.tile(cos, (1, NH))              # [S, NH*64]
    sin_rep = np.tile(sin, (1, NH))

    in_maps = []
    for c in range(NCORES):
        b = c // 4
        t0 = (c % 4) * R
        in_maps.append({
            "x_sh": np.ascontiguousarray(x[b, t0:t0 + R, :]),
            "cos_sh": np.ascontiguousarray(cos_rep[t0:t0 + R].reshape(NT, 128, NH * 64)),
            "sin_sh": np.ascontiguousarray(sin_rep[t0:t0 + R].reshape(NT, 128, NH * 64)),
            "attn_norm_w": anw, "ffn_norm_w": fnw,
            "wqkv_t": wqkv_t, "wo_t": wo_t, "wup_t": wup_t, "wdn_t": wdn_t,
        })
    return in_maps


def run(trace=False, **inputs):
    nc = _get_nc()
    in_maps = _host_inputs(**inputs)
    res = run_bass_kernel_spmd(nc, in_maps, list(range(NCORES)), trace=trace)
    out = np.empty((B, S, H), np.float32)
    for c in range(NCORES):
        b = c // 4
        t0 = (c % 4) * R
        out[b, t0:t0 + R, :] = res.results[c]["out_sh"]
    return out, res


def kernel(**inputs):
    out, _ = run(trace=False, **inputs)
    return out



# revision 9
# speedup vs baseline: 1.9038x; 1.9038x over previous
"""Trainium2 Bass kernel for nn_BlockAttentionResidual (block attention + BitNet MLP).

Sharding: sequence-block data parallelism. Block attention is independent per
512-token block, so each of the 8 cores owns 1024 contiguous tokens (2 blocks)
of one batch element and runs the ENTIRE layer on them with zero collectives.
  core c -> batch c//4, tokens [(c%4)*1024, (c%4+1)*1024)

v2 vs v1: all weight quantization moved to the host (ternary weights shipped as
bf16, exact), ffn_up shipped as fp16 (~= f32r precision at half the DMA bytes),
compact rope tables, act kept in SBUF, o_proj fused with ffn rmsnorm, K-side
dequant folded into the softmax exp scale.  Device work is just the matmul
stream + norms/quant/rope/softmax, structured to keep the PE dense.

Numerics: bitlinear matmuls run on integer-valued bf16 operands (act-quant ints
in [-128,127] x ternary weights are exact in bf16; PSUM accumulates fp32);
attention in bf16 with fp32 accum; softmax skips max-subtraction (|score·scale|
~ 3); ffn_up in fp16 (10 mantissa bits = f32r).
"""

import numpy as np
import ml_dtypes

import concourse.bass as bass
import concourse.mybir as mybir
import concourse.tile as tile
from concourse import bacc
from concourse.bass_utils import run_bass_kernel_spmd

F32 = mybir.dt.float32
BF16 = mybir.dt.bfloat16
F16 = mybir.dt.float16
AX = mybir.AxisListType
OP = mybir.AluOpType
ACTF = mybir.ActivationFunctionType

# model dims
H = 2048
NH = 16
HD = 128
INTER = 4096        # reference INTER = 2*H... ffn_up rows = 2*INTER = 8192
EPS = 1e-5
THETA = 10000.0
B, S = 2, 4096
NCORES = 8
R = 1024            # tokens per core (= 2 attention blocks of 512)
NT = R // 128       # 8 token tiles per core
MAGIC = np.float32(1.5 * 2 ** 23)   # fp32 round-to-nearest-even magic
SCALE_QK = float(HD ** -0.5)

# strip order for the interleaved qkv stream: per head-group g the q, k, v
# strips (cols g*512 | 2048+g*512 | 4096+g*512 of w_qkv^T)
QKV_ORDER = [0, 4, 8, 1, 5, 9, 2, 6, 10, 3, 7, 11]
UP_ORDER = [0, 8, 1, 9, 2, 10, 3, 11, 4, 12, 5, 13, 6, 14, 7, 15]


def build_program(skip_anw, skip_fnw):
    nc = bacc.Bacc(None, target_bir_lowering=False)

    # ---- I/O ----
    x_in = nc.declare_dram_parameter("x_sh", [R, H], F32, isOutput=False)
    cos_in = nc.declare_dram_parameter("cosb", [NT, 128, 64], F32, isOutput=False)
    sin_in = nc.declare_dram_parameter("sinb", [NT, 128, 64], F32, isOutput=False)
    anw_in = nc.declare_dram_parameter("attn_norm_w", [H], F32, isOutput=False)
    fnw_in = nc.declare_dram_parameter("ffn_norm_w", [H], F32, isOutput=False)
    dqs_in = nc.declare_dram_parameter("dqs", [4], F32, isOutput=False)
    wqkv_in = nc.declare_dram_parameter("wqkv3", [12, 16, 128, 512], BF16, isOutput=False)
    wo_in = nc.declare_dram_parameter("wo3", [4, 16, 128, 512], BF16, isOutput=False)
    wup_in = nc.declare_dram_parameter("wup_h", [16, 16, 128, 512], F16, isOutput=False)
    wdn_in = nc.declare_dram_parameter("wdn3", [4, 32, 128, 512], BF16, isOutput=False)
    out_d = nc.declare_dram_parameter("out_sh", [R, H], F32, isOutput=True)

    # ---- internal DRAM scratch ----
    x1_d = nc.dram_tensor("x1_d", [NT, 128, H], F32)

    with tile.TileContext(nc) as tc:
        perm = tc.alloc_tile_pool(name="perm", bufs=1)
        magic_t = perm.tile([128, 1], F32)
        nc.vector.memset(magic_t[:], float(MAGIC))
        magic_ap = magic_t[:]
        cos_st = perm.tile([128, NT, 4, 64], F32)
        sin_st = perm.tile([128, NT, 4, 64], F32)
        dq_b = perm.tile([128, 4], F32)
        dqa_all = perm.tile([128, NT], F32)    # per-token dequant for q/v (amc*m_qkv/127)
        e_scale = perm.tile([128, NT], F32)    # exp scale per key tile = dqa*HD^-.5
        dqc_all = perm.tile([128, NT], F32)
        dqact_all = perm.tile([128, NT], F32)

        # ------------ constants: rope tables, dequant scalars, norm weights --
        with tc.tile_pool(name="p0", bufs=2) as p0:
            cs = p0.tile([128, NT, 64], F32, tag="cscmp")
            nc.sync.dma_start(cs[:], cos_in.rearrange("t p f -> p t f"))
            for c in range(4):
                nc.vector.tensor_copy(cos_st[:, :, c, :], cs[:])
            sn = p0.tile([128, NT, 64], F32, tag="cscmp")
            nc.sync.dma_start(sn[:], sin_in.rearrange("t p f -> p t f"))
            for c in range(4):
                nc.vector.tensor_copy(sin_st[:, :, c, :], sn[:])
        ap0 = dqs_in[:]
        nc.gpsimd.dma_start(out=dq_b[:], in_=bass.AP(
            tensor=ap0.tensor, offset=ap0.offset, ap=[[0, 128]] + list(ap0.ap)))

        nwpool = tc.alloc_tile_pool(name="nw", bufs=1)
        if not skip_anw:
            anw_b = nwpool.tile([128, H], F32, tag="anw")
            ap0 = anw_in[:]
            nc.gpsimd.dma_start(out=anw_b[:], in_=bass.AP(
                tensor=ap0.tensor, offset=ap0.offset, ap=[[0, 128]] + list(ap0.ap)))
        if not skip_fnw:
            fnw_b = nwpool.tile([128, H], F32, tag="fnw")
            ap0 = fnw_in[:]
            nc.gpsimd.dma_start(out=fnw_b[:], in_=bass.AP(
                tensor=ap0.tensor, offset=ap0.offset, ap=[[0, 128]] + list(ap0.ap)))

        # ------------ N1: attn rmsnorm + act-quant + transpose ---------------
        slotA = tc.alloc_tile_pool(name="slotA", bufs=1)
        xqT = slotA.tile([128, 16, NT, 128], BF16, tag="xqT")
        with tc.tile_pool(name="npool", bufs=2) as npool:
            for t in range(NT):
                xt = npool.tile([128, H], F32, tag="xt")
                nc.sync.dma_start(xt[:], x_in[t * 128:(t + 1) * 128, :])
                ssq = npool.tile([128, 1], F32, tag="ssq")
                junk = npool.tile([128, H], BF16, tag="njunk")
                nc.scalar.activation(junk[:], xt[:], ACTF.Square, accum_out=ssq[:])
                msq = npool.tile([128, 1], F32, tag="msq")
                nc.vector.tensor_scalar(msq[:], ssq[:], 1.0 / H, EPS, OP.mult, OP.add)
                sd = npool.tile([128, 1], F32, tag="sd")
                nc.scalar.activation(sd[:], msq[:], ACTF.Sqrt)
                rstd = npool.tile([128, 1], F32, tag="rstd")
                nc.vector.reciprocal(rstd[:], sd[:])
                if skip_anw:
                    amax = npool.tile([128, 1], F32, tag="amax")
                    nc.vector.tensor_reduce(amax[:], xt[:], AX.X, OP.max,
                                            apply_absolute_value=True)
                    amc = npool.tile([128, 1], F32, tag="amc")
                    nc.vector.tensor_scalar(amc[:], amax[:], rstd[:], 1e-5,
                                            OP.mult, OP.max)
                    rec = npool.tile([128, 1], F32, tag="rec")
                    nc.vector.reciprocal(rec[:], amc[:])
                    smul = npool.tile([128, 1], F32, tag="smul")
                    nc.vector.tensor_scalar(smul[:], rec[:], rstd[:], 127.0,
                                            OP.mult, OP.mult)
                    src_ap = xt[:]
                else:
                    h_t = npool.tile([128, H], F32, tag="h_t")
                    nc.vector.tensor_scalar_mul(h_t[:], xt[:], rstd[:])
                    nc.vector.tensor_tensor(h_t[:], h_t[:], anw_b[:], OP.mult)
                    amax = npool.tile([128, 1], F32, tag="amax")
                    nc.vector.tensor_reduce(amax[:], h_t[:], AX.X, OP.max,
                                            apply_absolute_value=True)
                    amc = npool.tile([128, 1], F32, tag="amc")
                    nc.vector.tensor_scalar_max(amc[:], amax[:], 1e-5)
                    rec = npool.tile([128, 1], F32, tag="rec")
                    nc.vector.reciprocal(rec[:], amc[:])
                    smul = npool.tile([128, 1], F32, tag="smul")
                    nc.vector.tensor_scalar_mul(smul[:], rec[:], 127.0)
                    src_ap = h_t[:]
                nc.vector.tensor_scalar(dqa_all[:, t:t + 1], amc[:], dq_b[:, 0:1],
                                        None, OP.mult)
                nc.vector.tensor_scalar_mul(e_scale[:, t:t + 1],
                                            dqa_all[:, t:t + 1], SCALE_QK)
                mg = npool.tile([128, H], F32, tag="mg")
                nc.scalar.activation(mg[:], src_ap, ACTF.Identity,
                                     bias=magic_ap, scale=smul[:])
                xq = npool.tile([128, H], BF16, tag="xq")
                nc.vector.tensor_scalar_sub(xq[:], mg[:], float(MAGIC))
                nc.sync.dma_start_transpose(xqT[:, :, t, :], xq[:])

        # ------------ QKV + rope + block attention, per 4-head group ---------
        ctxA = tc.alloc_tile_pool(name="ctxA", bufs=1, side="right")
        ctx_sb = ctxA.tile([128, NT, NH, 128], BF16, tag="ctx")
        qkarena = tc.alloc_tile_pool(name="qkarena", bufs=2)
        with tc.tile_pool(name="wqpool", bufs=2) as wqpool, \
             tc.tile_pool(name="rpool", bufs=2) as rpool, \
             tc.tile_pool(name="apool", bufs=2) as apool, \
             tc.tile_pool(name="qkv_psum", bufs=3, space="PSUM") as psum_mm, \
             tc.tile_pool(name="at_psum", bufs=2, space="PSUM") as psum_at, \
             tc.tile_pool(name="ctx_psum", bufs=2, space="PSUM") as psum_cx:
            for g in range(4):
                qTg = qkarena.tile([128, 4, NT, 128], BF16, tag="qTg")
                kTg = qkarena.tile([128, 4, NT, 128], BF16, tag="kTg")
                vg = qkarena.tile([128, NT, 4, 132], BF16, tag="vg")
                nc.vector.memset(vg[:, :, :, 128:129], 1.0)
                for sub in range(3):     # q, k, v strips of this head group
                    wst = wqpool.tile([128, 16, 512], BF16, tag="wst")
                    nc.sync.dma_start(wst[:], wqkv_in[3 * g + sub]
                                      .rearrange("k p f -> p k f"))
                    for t in range(NT):
                        ps = psum_mm.tile([128, 512], F32, tag="ps_qkv")
                        for kk in range(16):
                            nc.tensor.matmul(ps[:], xqT[:, kk, t, :],
                                             wst[:, kk, :],
                                             start=(kk == 0), stop=(kk == 15))
                        dqa_ap = dqa_all[:, t:t + 1]
                        if sub == 2:     # V: dequant into the augmented tile
                            nc.vector.tensor_scalar_mul(
                                vg[:, t, :, 0:128],
                                ps[:].rearrange("p (c f) -> p c f", c=4), dqa_ap)
                        else:
                            cosd = cos_st[:, t, :, :]
                            sind = sin_st[:, t, :, :]
                            rot = rpool.tile([128, 4, 128], BF16, tag="rot")
                            if sub == 0:    # Q: dequant then rope (gpsimd ok)
                                qsc = rpool.tile([128, 512], F32, tag="qsc")
                                nc.vector.tensor_scalar_mul(qsc[:], ps[:], dqa_ap)
                                q3 = qsc[:].rearrange("p (c f) -> p c f", c=4)
                                p1, p2 = q3[:, :, 0:64], q3[:, :, 64:128]
                                t1 = rpool.tile([128, 4, 64], F32, tag="rt1")
                                t2 = rpool.tile([128, 4, 64], F32, tag="rt2")
                                nc.vector.tensor_tensor(t1[:], p1, cosd, OP.mult)
                                nc.gpsimd.tensor_tensor(t2[:], p2, sind, OP.mult)
                                nc.vector.tensor_tensor(rot[:, :, 0:64], t1[:],
                                                        t2[:], OP.subtract)
                                nc.vector.tensor_tensor(t1[:], p2, cosd, OP.mult)
                                nc.gpsimd.tensor_tensor(t2[:], p1, sind, OP.mult)
                                nc.vector.tensor_tensor(rot[:, :, 64:128], t1[:],
                                                        t2[:], OP.add)
                            else:           # K: rope raw ints from PSUM (DVE only;
                                            # dequant folds into the exp scale)
                                k3 = ps[:].rearrange("p (c f) -> p c f", c=4)
                                p1, p2 = k3[:, :, 0:64], k3[:, :, 64:128]
                                t1 = rpool.tile([128, 4, 64], F32, tag="rt1")
                                t2 = rpool.tile([128, 4, 64], F32, tag="rt2")
                                nc.vector.tensor_tensor(t1[:], p1, cosd, OP.mult)
                                nc.vector.tensor_tensor(t2[:], p2, sind, OP.mult)
                                nc.vector.tensor_tensor(rot[:, :, 0:64], t1[:],
                                                        t2[:], OP.subtract)
                                nc.vector.tensor_tensor(t1[:], p2, cosd, OP.mult)
                                nc.vector.tensor_tensor(t2[:], p1, sind, OP.mult)
                                nc.vector.tensor_tensor(rot[:, :, 64:128], t1[:],
                                                        t2[:], OP.add)
                            dst = qTg if sub == 0 else kTg
                            nc.sync.dma_start_transpose(
                                dst[:, :, t, :],
                                rot[:].rearrange("p c f -> p (c f)"))
                # block attention for heads 4g..4g+4
                for blk in range(2):
                    for hh in range(4):
                        expT = [None] * 4
                        for kt in range(4):
                            qn = 512 - kt * 128
                            pss = psum_at.tile([128, 512], F32, tag="ps_sc")
                            nc.tensor.matmul(
                                pss[:, 0:qn],
                                kTg[:, hh, blk * 4 + kt, :],
                                qTg[:, hh, blk * 4:(blk + 1) * 4, :]
                                .rearrange("p c f -> p (c f)")[:, kt * 128:512],
                                start=True, stop=True)
                            ex = apool.tile([128, 512], BF16, tag=f"expT{kt}")
                            nc.scalar.activation(
                                ex[:, 0:qn], pss[:, 0:qn], ACTF.Exp,
                                scale=e_scale[:, blk * 4 + kt:blk * 4 + kt + 1])
                            nc.gpsimd.affine_select(
                                out=ex[:, 0:128], in_=ex[:, 0:128],
                                compare_op=OP.is_ge, fill=0.0,
                                base=0, pattern=[[1, 128]], channel_multiplier=-1)
                            expT[kt] = ex
                        for qt in range(4):
                            psc = psum_cx.tile([128, 132], F32, tag="ps_ctx")
                            for kt in range(qt + 1):
                                nc.tensor.matmul(
                                    psc[:, 0:129],
                                    expT[kt][:, (qt - kt) * 128:(qt - kt) * 128 + 128],
                                    vg[:, blk * 4 + kt, hh, 0:129],
                                    start=(kt == 0), stop=(kt == qt))
                            rl = apool.tile([128, 1], F32, tag="rl")
                            nc.vector.reciprocal(rl[:], psc[:, 128:129])
                            nc.vector.tensor_scalar_mul(
                                ctx_sb[:, blk * 4 + qt, 4 * g + hh, :],
                                psc[:, 0:128], rl[:])
        qkarena.release()
        slotA.release()

        # ------------ ctx act-quant + transpose ------------------------------
        ctxqA = tc.alloc_tile_pool(name="ctxqA", bufs=1)
        ctxqT = ctxqA.tile([128, 16, NT, 128], BF16, tag="ctxqT")
        with tc.tile_pool(name="cqpool", bufs=2) as cqpool:
            for t in range(NT):
                src = ctx_sb[:, t, :, :].rearrange("p c f -> p (c f)")
                amax = cqpool.tile([128, 1], F32, tag="camax")
                nc.vector.tensor_reduce(amax[:], src, AX.X, OP.max,
                                        apply_absolute_value=True)
                amc = cqpool.tile([128, 1], F32, tag="camc")
                nc.vector.tensor_scalar_max(amc[:], amax[:], 1e-5)
                rec = cqpool.tile([128, 1], F32, tag="crec")
                nc.vector.reciprocal(rec[:], amc[:])
                s_c = cqpool.tile([128, 1], F32, tag="cs")
                nc.vector.tensor_scalar_mul(s_c[:], rec[:], 127.0)
                nc.vector.tensor_scalar(dqc_all[:, t:t + 1], amc[:], dq_b[:, 1:2],
                                        None, OP.mult)
                mg = cqpool.tile([128, H], F32, tag="cmg")
                nc.scalar.activation(mg[:], src, ACTF.Identity,
                                     bias=magic_ap, scale=s_c[:])
                cq = cqpool.tile([128, H], BF16, tag="cq")
                nc.vector.tensor_scalar_sub(cq[:], mg[:], float(MAGIC))
                nc.sync.dma_start_transpose(ctxqT[:, :, t, :], cq[:])
        ctxA.release()

        # ------------ o_proj + residual + ffn rmsnorm (fused per tile) -------
        hnA = tc.alloc_tile_pool(name="hnA", bufs=1, side="right")
        hnT = hnA.tile([128, 16, NT, 128], F16, tag="hnT")
        with tc.tile_pool(name="wopool", bufs=1) as wopool, \
             tc.tile_pool(name="opool", bufs=3) as opool, \
             tc.tile_pool(name="o_psum", bufs=3, space="PSUM") as psum_o:
            wot = wopool.tile([128, 16, 4, 512], BF16, tag="wot")
            for c4 in range(4):
                nc.sync.dma_start(wot[:, :, c4, :],
                                  wo_in[c4].rearrange("k p f -> p k f"))
            for t in range(NT):
                x1t = opool.tile([128, H], F32, tag="x1t")
                for nn in range(4):
                    ps = psum_o.tile([128, 512], F32, tag="ps_o")
                    for kk in range(16):
                        nc.tensor.matmul(ps[:], ctxqT[:, kk, t, :],
                                         wot[:, kk, nn, :],
                                         start=(kk == 0), stop=(kk == 15))
                    xs = opool.tile([128, 512], F32, tag="xs")
                    nc.scalar.dma_start(xs[:], x_in[t * 128:(t + 1) * 128,
                                                    nn * 512:(nn + 1) * 512])
                    tmp = opool.tile([128, 512], F32, tag="o_tmp")
                    nc.vector.tensor_scalar_mul(tmp[:], ps[:], dqc_all[:, t:t + 1])
                    nc.vector.tensor_tensor(x1t[:, nn * 512:(nn + 1) * 512],
                                            tmp[:], xs[:], OP.add)
                nc.sync.dma_start(x1_d[t, :, :], x1t[:])
                # ffn rmsnorm on the in-SBUF x1 tile
                ssq = opool.tile([128, 1], F32, tag="ssq2")
                junk = opool.tile([128, H], BF16, tag="njunk2")
                nc.scalar.activation(junk[:], x1t[:], ACTF.Square, accum_out=ssq[:])
                msq = opool.tile([128, 1], F32, tag="msq2")
                nc.vector.tensor_scalar(msq[:], ssq[:], 1.0 / H, EPS, OP.mult, OP.add)
                sd = opool.tile([128, 1], F32, tag="sd2")
                nc.scalar.activation(sd[:], msq[:], ACTF.Sqrt)
                rstd = opool.tile([128, 1], F32, tag="rstd2")
                nc.vector.reciprocal(rstd[:], sd[:])
                hn16 = opool.tile([128, H], F16, tag="hn16")
                if skip_fnw:
                    nc.scalar.activation(hn16[:], x1t[:], ACTF.Identity,
                                         scale=rstd[:])
                else:
                    hn = opool.tile([128, H], F32, tag="hnf")
                    nc.vector.tensor_scalar_mul(hn[:], x1t[:], rstd[:])
                    nc.vector.tensor_tensor(hn16[:], hn[:], fnw_b[:], OP.mult)
                nc.sync.dma_start_transpose(hnT[:, :, t, :], hn16[:])
        ctxqA.release()

        # ------------ ffn up (fp16) + silu*val -> act in SBUF ----------------
        actA = tc.alloc_tile_pool(name="actA", bufs=1)
        act_t = []
        for t in range(NT):
            a_tile = actA.tile([128, INTER], F16, tag=f"act{t}")
            act_t.append(a_tile)
        with tc.tile_pool(name="upool", bufs=2) as upool, \
             tc.tile_pool(name="fpool", bufs=3) as fpool, \
             tc.tile_pool(name="up_psum", bufs=2, space="PSUM") as psum_up, \
             tc.tile_pool(name="upv_psum", bufs=2, space="PSUM") as psum_upv:
            for i in range(8):   # paired gate/val strips of 512
                wg = upool.tile([128, 16, 512], F16, tag="wg")
                nc.sync.dma_start(wg[:], wup_in[2 * i].rearrange("k p f -> p k f"))
                wv = upool.tile([128, 16, 512], F16, tag="wv")
                nc.sync.dma_start(wv[:], wup_in[2 * i + 1].rearrange("k p f -> p k f"))
                for t in range(NT):
                    psg = psum_up.tile([128, 512], F32, tag="ps_g")
                    for kk in range(16):
                        nc.tensor.matmul(psg[:], hnT[:, kk, t, :], wg[:, kk, :],
                                         start=(kk == 0), stop=(kk == 15))
                    psv = psum_upv.tile([128, 512], F32, tag="ps_v")
                    for kk in range(16):
                        nc.tensor.matmul(psv[:], hnT[:, kk, t, :], wv[:, kk, :],
                                         start=(kk == 0), stop=(kk == 15))
                    sgm = fpool.tile([128, 512], F32, tag="sgm")
                    nc.scalar.activation(sgm[:], psg[:], ACTF.Sigmoid)
                    sg = fpool.tile([128, 512], F32, tag="sg")
                    nc.vector.tensor_tensor(sg[:], sgm[:], psg[:], OP.mult)
                    nc.vector.tensor_tensor(act_t[t][:, i * 512:(i + 1) * 512],
                                            sg[:], psv[:], OP.mult)
        hnA.release()

        # ------------ act quant + transpose ----------------------------------
        aqA = tc.alloc_tile_pool(name="aqA", bufs=1, side="right")
        actqT = aqA.tile([128, 32, NT, 128], BF16, tag="actqT")
        with tc.tile_pool(name="aqpool", bufs=2) as aqpool:
            for t in range(NT):
                amax = aqpool.tile([128, 1], F32, tag="aamax")
                nc.vector.tensor_reduce(amax[:], act_t[t][:], AX.X, OP.max,
                                        apply_absolute_value=True)
                amc = aqpool.tile([128, 1], F32, tag="aamc")
                nc.vector.tensor_scalar_max(amc[:], amax[:], 1e-5)
                rec = aqpool.tile([128, 1], F32, tag="arec")
                nc.vector.reciprocal(rec[:], amc[:])
                s_a = aqpool.tile([128, 1], F32, tag="as")
                nc.vector.tensor_scalar_mul(s_a[:], rec[:], 127.0)
                nc.vector.tensor_scalar(dqact_all[:, t:t + 1], amc[:], dq_b[:, 2:3],
                                        None, OP.mult)
                for half in range(2):
                    sl = slice(half * 2048, (half + 1) * 2048)
                    mg = aqpool.tile([128, 2048], F32, tag="amg")
                    nc.scalar.activation(mg[:], act_t[t][:, sl], ACTF.Identity,
                                         bias=magic_ap, scale=s_a[:])
                    aq = aqpool.tile([128, 2048], BF16, tag="aq")
                    nc.vector.tensor_scalar_sub(aq[:], mg[:], float(MAGIC))
                    nc.sync.dma_start_transpose(
                        actqT[:, half * 16:(half + 1) * 16, t, :], aq[:])
        actA.release()

        # ------------ ffn down + residual -> out -----------------------------
        with tc.tile_pool(name="dpool", bufs=2) as dpool, \
             tc.tile_pool(name="dopool", bufs=3) as dopool, \
             tc.tile_pool(name="dn_psum", bufs=3, space="PSUM") as psum_dn:
            for nn in range(4):
                wdt = dpool.tile([128, 32, 512], BF16, tag="wdt")
                nc.sync.dma_start(wdt[:], wdn_in[nn].rearrange("k p f -> p k f"))
                for t in range(NT):
                    ps = psum_dn.tile([128, 512], F32, tag="ps_dn")
                    for kk in range(32):
                        nc.tensor.matmul(ps[:], actqT[:, kk, t, :], wdt[:, kk, :],
                                         start=(kk == 0), stop=(kk == 31))
                    x1s = dopool.tile([128, 512], F32, tag="x1s")
                    nc.scalar.dma_start(x1s[:], x1_d[t, :, nn * 512:(nn + 1) * 512])
                    tmp = dopool.tile([128, 512], F32, tag="d_tmp")
                    nc.vector.tensor_scalar_mul(tmp[:], ps[:], dqact_all[:, t:t + 1])
                    ot = dopool.tile([128, 512], F32, tag="ot")
                    nc.vector.tensor_tensor(ot[:], tmp[:], x1s[:], OP.add)
                    nc.sync.dma_start(out_d[t * 128:(t + 1) * 128,
                                            nn * 512:(nn + 1) * 512], ot[:])
        aqA.release()
        nwpool.release()
        perm.release()

    nc.compile()
    return nc


_NC_CACHE = {}


def _get_nc(skip_anw, skip_fnw):
    key = (skip_anw, skip_fnw)
    if key not in _NC_CACHE:
        _NC_CACHE[key] = build_program(skip_anw, skip_fnw)
    return _NC_CACHE[key]


def _ternarize(w):
    """Exact reference _weight_quant: per-tensor ternary + fp32 scale."""
    w = np.asarray(w, np.float32)
    m = np.float32(max(np.float32(np.mean(np.abs(w))), np.float32(1e-5)))
    scale = np.float32(1.0) / m
    t = np.clip(np.rint(w * scale), -1, 1)
    return t, np.float32(m)


def _host_inputs(x, attn_norm_w, ffn_norm_w, qkv_w, o_w, ffn_up_w, ffn_down_w):
    x = np.ascontiguousarray(np.asarray(x, np.float32))
    anw = np.ascontiguousarray(np.asarray(attn_norm_w, np.float32))
    fnw = np.ascontiguousarray(np.asarray(ffn_norm_w, np.float32))

    tq, m_qkv = _ternarize(np.asarray(qkv_w, np.float32).T)       # [2048, 6144]
    to, m_o = _ternarize(np.asarray(o_w, np.float32).T)           # [2048, 2048]
    td, m_dn = _ternarize(np.asarray(ffn_down_w, np.float32).T)   # [4096, 2048]

    wqkv3 = np.ascontiguousarray(
        tq.reshape(16, 128, 12, 512).transpose(2, 0, 1, 3)[QKV_ORDER]
    ).astype(ml_dtypes.bfloat16)
    wo3 = np.ascontiguousarray(
        to.reshape(16, 128, 4, 512).transpose(2, 0, 1, 3)
    ).astype(ml_dtypes.bfloat16)
    wdn3 = np.ascontiguousarray(
        td.reshape(32, 128, 4, 512).transpose(2, 0, 1, 3)
    ).astype(ml_dtypes.bfloat16)
    wup = np.asarray(ffn_up_w, np.float32).T                      # [2048, 8192]
    wup_h = np.ascontiguousarray(
        wup.reshape(16, 128, 16, 512).transpose(2, 0, 1, 3)[UP_ORDER]
    ).astype(np.float16)

    dqs = np.array([m_qkv / 127.0, m_o / 127.0, m_dn / 127.0, 0.0], np.float32)

    inv = 1.0 / (THETA ** (np.arange(0, HD, 2, dtype=np.float32) / HD))
    tpos = np.arange(S, dtype=np.float32)
    fr = np.outer(tpos, inv)                     # [S, 64]
    cosf = np.cos(fr).astype(np.float32)
    sinf = np.sin(fr).astype(np.float32)

    in_maps = []
    for c in range(NCORES):
        b = c // 4
        t0 = (c % 4) * R
        in_maps.append({
            "x_sh": np.ascontiguousarray(x[b, t0:t0 + R, :]),
            "cosb": np.ascontiguousarray(cosf[t0:t0 + R].reshape(NT, 128, 64)),
            "sinb": np.ascontiguousarray(sinf[t0:t0 + R].reshape(NT, 128, 64)),
            "attn_norm_w": anw, "ffn_norm_w": fnw, "dqs": dqs,
            "wqkv3": wqkv3, "wo3": wo3, "wup_h": wup_h, "wdn3": wdn3,
        })
    skip_anw = bool(np.all(anw == 1.0))
    skip_fnw = bool(np.all(fnw == 1.0))
    return in_maps, skip_anw, skip_fnw


def run(trace=False, **inputs):
    in_maps, skip_anw, skip_fnw = _host_inputs(**inputs)
    nc = _get_nc(skip_anw, skip_fnw)
    res = run_bass_kernel_spmd(nc, in_maps, list(range(NCORES)), trace=trace)
    out = np.empty((B, S, H), np.float32)
    for c in range(NCORES):
        b = c // 4
        t0 = (c % 4) * R
        out[b, t0:t0 + R, :] = res.results[c]["out_sh"]
    return out, res


def kernel(**inputs):
    out, _ = run(trace=False, **inputs)
    return out


# revision 11
# speedup vs baseline: 2.1729x; 1.1413x over previous
"""Trainium2 Bass kernel for nn_BlockAttentionResidual (block attention + BitNet MLP).

Sharding: sequence-block data parallelism. Block attention is independent per
512-token block, so each of the 8 cores owns 1024 contiguous tokens (2 blocks)
of one batch element and runs the ENTIRE layer on them with zero collectives.
  core c -> batch c//4, tokens [(c%4)*1024, (c%4+1)*1024)

All weight quantization happens on the host (ternary weights shipped as bf16,
exact), ffn_up ships as fp16 (~= f32r precision at half the DMA bytes); compact
rope tables; o_proj fused with the ffn rmsnorm; K-side dequant folded into the
softmax exp scale.  Pool lifetimes are staged so each phase's weights prefetch
during the previous phase and the quant chains overlap the matmul stream.

Numerics: bitlinear matmuls run on integer-valued bf16 operands (act-quant ints
in [-128,127] x ternary weights are exact in bf16; PSUM accumulates fp32);
attention in bf16 with fp32 accum; softmax skips max-subtraction (|score*scale|
~ 3); ffn_up in fp16 (10 mantissa bits = f32r).
"""

import numpy as np
import ml_dtypes

import concourse.bass as bass
import concourse.mybir as mybir
import concourse.tile as tile
from concourse import bacc
from concourse.bass_utils import run_bass_kernel_spmd

F32 = mybir.dt.float32
BF16 = mybir.dt.bfloat16
F16 = mybir.dt.float16
AX = mybir.AxisListType
OP = mybir.AluOpType
ACTF = mybir.ActivationFunctionType

# model dims
H = 2048
NH = 16
HD = 128
INTER = 4096        # reference INTER = 2*H; ffn_up rows = 2*INTER = 8192
EPS = 1e-5
THETA = 10000.0
B, S = 2, 4096
NCORES = 8
R = 1024            # tokens per core (= 2 attention blocks of 512)
NT = R // 128       # 8 token tiles per core
MAGIC = np.float32(1.5 * 2 ** 23)   # fp32 round-to-nearest-even magic
SCALE_QK = float(HD ** -0.5)

# strip order for the interleaved qkv stream: per head-group g the q, k, v
# strips (cols g*512 | 2048+g*512 | 4096+g*512 of w_qkv^T)
QKV_ORDER = [0, 4, 8, 1, 5, 9, 2, 6, 10, 3, 7, 11]
UP_ORDER = [0, 8, 1, 9, 2, 10, 3, 11, 4, 12, 5, 13, 6, 14, 7, 15]


def build_program(skip_anw, skip_fnw):
    nc = bacc.Bacc(None, target_bir_lowering=False)

    # ---- I/O ----
    x_in = nc.declare_dram_parameter("x_sh", [R, H], F32, isOutput=False)
    cos_in = nc.declare_dram_parameter("cosb", [NT, 128, 64], F32, isOutput=False)
    sin_in = nc.declare_dram_parameter("sinb", [NT, 128, 64], F32, isOutput=False)
    anw_in = nc.declare_dram_parameter("attn_norm_w", [H], F32, isOutput=False)
    fnw_in = nc.declare_dram_parameter("ffn_norm_w", [H], F32, isOutput=False)
    dqs_in = nc.declare_dram_parameter("dqs", [4], F32, isOutput=False)
    wqkv_in = nc.declare_dram_parameter("wqkv3", [12, 16, 128, 512], BF16, isOutput=False)
    wo_in = nc.declare_dram_parameter("wo3", [4, 16, 128, 512], BF16, isOutput=False)
    wup_in = nc.declare_dram_parameter("wup_h", [16, 16, 128, 512], F16, isOutput=False)
    wdn_in = nc.declare_dram_parameter("wdn3", [4, 32, 128, 512], BF16, isOutput=False)
    out_d = nc.declare_dram_parameter("out_sh", [R, H], F32, isOutput=True)

    # ---- internal DRAM scratch ----
    x1_d = nc.dram_tensor("x1_d", [NT, 128, H], F32)
    act_d = nc.dram_tensor("act_d", [NT, 128, INTER], F16)

    with tile.TileContext(nc) as tc:
        perm = tc.alloc_tile_pool(name="perm", bufs=1)
        magic_t = perm.tile([128, 1], F32)
        nc.vector.memset(magic_t[:], float(MAGIC))
        magic_ap = magic_t[:]
        dq_b = perm.tile([128, 4], F32)
        dqa_all = perm.tile([128, NT], F32)    # per-token dequant for q/v
        e_scale = perm.tile([128, NT], F32)    # exp scale per key tile
        dqc_all = perm.tile([128, NT], F32)
        dqact_all = perm.tile([128, NT], F32)
        ap0 = dqs_in[:]
        nc.gpsimd.dma_start(out=dq_b[:], in_=bass.AP(
            tensor=ap0.tensor, offset=ap0.offset, ap=[[0, 128]] + list(ap0.ap)))

        nwpool = tc.alloc_tile_pool(name="nw", bufs=1)
        if not skip_anw:
            anw_b = nwpool.tile([128, H], F32, tag="anw")
            ap0 = anw_in[:]
            nc.gpsimd.dma_start(out=anw_b[:], in_=bass.AP(
                tensor=ap0.tensor, offset=ap0.offset, ap=[[0, 128]] + list(ap0.ap)))
        if not skip_fnw:
            fnw_b = nwpool.tile([128, H], F32, tag="fnw")
            ap0 = fnw_in[:]
            nc.gpsimd.dma_start(out=fnw_b[:], in_=bass.AP(
                tensor=ap0.tensor, offset=ap0.offset, ap=[[0, 128]] + list(ap0.ap)))

        # rope tables (freed after attention)
        csA = tc.alloc_tile_pool(name="csA", bufs=1)
        cos_st = csA.tile([128, NT, 4, 64], F32, tag="cosst")
        sin_st = csA.tile([128, NT, 4, 64], F32, tag="sinst")
        with tc.tile_pool(name="p0", bufs=2) as p0:
            cs = p0.tile([128, NT, 64], F32, tag="cscmp")
            nc.sync.dma_start(cs[:], cos_in.rearrange("t p f -> p t f"))
            for c in range(4):
                nc.vector.tensor_copy(cos_st[:, :, c, :], cs[:])
            sn = p0.tile([128, NT, 64], F32, tag="cscmp")
            nc.sync.dma_start(sn[:], sin_in.rearrange("t p f -> p t f"))
            for c in range(4):
                nc.vector.tensor_copy(sin_st[:, :, c, :], sn[:])

        ctxA = tc.alloc_tile_pool(name="ctxA", bufs=1, side="right")
        ctx_sb = ctxA.tile([128, NT, NH, 128], BF16, tag="ctx")

        # ------------ N1: attn rmsnorm + act-quant + transpose ---------------
        slotA = tc.alloc_tile_pool(name="slotA", bufs=1)
        xqT = slotA.tile([128, 16, NT, 128], BF16, tag="xqT")
        with tc.tile_pool(name="npool", bufs=2) as npool:
            for t in range(NT):
                xt = npool.tile([128, H], F32, tag="xt")
                nc.sync.dma_start(xt[:], x_in[t * 128:(t + 1) * 128, :])
                ssq = npool.tile([128, 1], F32, tag="ssq")
                junk = npool.tile([128, H], BF16, tag="njunk")
                nc.scalar.activation(junk[:], xt[:], ACTF.Square, accum_out=ssq[:])
                msq = npool.tile([128, 1], F32, tag="msq")
                nc.vector.tensor_scalar(msq[:], ssq[:], 1.0 / H, EPS, OP.mult, OP.add)
                sd = npool.tile([128, 1], F32, tag="sd")
                nc.scalar.activation(sd[:], msq[:], ACTF.Sqrt)
                rstd = npool.tile([128, 1], F32, tag="rstd")
                nc.vector.reciprocal(rstd[:], sd[:])
                if skip_anw:
                    amax = npool.tile([128, 1], F32, tag="amax")
                    nc.vector.tensor_reduce(amax[:], xt[:], AX.X, OP.max,
                                            apply_absolute_value=True)
                    amc = npool.tile([128, 1], F32, tag="amc")
                    nc.vector.tensor_scalar(amc[:], amax[:], rstd[:], 1e-5,
                                            OP.mult, OP.max)
                    rec = npool.tile([128, 1], F32, tag="rec")
                    nc.vector.reciprocal(rec[:], amc[:])
                    smul = npool.tile([128, 1], F32, tag="smul")
                    nc.vector.tensor_scalar(smul[:], rec[:], rstd[:], 127.0,
                                            OP.mult, OP.mult)
                    src_ap = xt[:]
                else:
                    h_t = npool.tile([128, H], F32, tag="h_t")
                    nc.vector.tensor_scalar_mul(h_t[:], xt[:], rstd[:])
                    nc.vector.tensor_tensor(h_t[:], h_t[:], anw_b[:], OP.mult)
                    amax = npool.tile([128, 1], F32, tag="amax")
                    nc.vector.tensor_reduce(amax[:], h_t[:], AX.X, OP.max,
                                            apply_absolute_value=True)
                    amc = npool.tile([128, 1], F32, tag="amc")
                    nc.vector.tensor_scalar_max(amc[:], amax[:], 1e-5)
                    rec = npool.tile([128, 1], F32, tag="rec")
                    nc.vector.reciprocal(rec[:], amc[:])
                    smul = npool.tile([128, 1], F32, tag="smul")
                    nc.vector.tensor_scalar_mul(smul[:], rec[:], 127.0)
                    src_ap = h_t[:]
                nc.vector.tensor_scalar(dqa_all[:, t:t + 1], amc[:], dq_b[:, 0:1],
                                        None, OP.mult)
                nc.vector.tensor_scalar_mul(e_scale[:, t:t + 1],
                                            dqa_all[:, t:t + 1], SCALE_QK)
                mg = npool.tile([128, H], F32, tag="mg")
                nc.scalar.activation(mg[:], src_ap, ACTF.Identity,
                                     bias=magic_ap, scale=smul[:])
                xq = npool.tile([128, H], BF16, tag="xq")
                nc.vector.tensor_scalar_sub(xq[:], mg[:], float(MAGIC))
                nc.sync.dma_start_transpose(xqT[:, :, t, :], xq[:])

        # ------------ QKV + rope + block attention, per 4-head group ---------
        qkarena = tc.alloc_tile_pool(name="qkarena", bufs=2)
        with tc.tile_pool(name="wqpool", bufs=2, side="right") as wqpool, \
             tc.tile_pool(name="rpool", bufs=2) as rpool, \
             tc.tile_pool(name="apool", bufs=2) as apool, \
             tc.tile_pool(name="qkv_psum", bufs=3, space="PSUM") as psum_mm, \
             tc.tile_pool(name="at_psum", bufs=2, space="PSUM") as psum_at, \
             tc.tile_pool(name="ctx_psum", bufs=2, space="PSUM") as psum_cx:
            for g in range(4):
                qTg = qkarena.tile([128, 4, NT, 128], BF16, tag="qTg")
                kTg = qkarena.tile([128, 4, NT, 128], BF16, tag="kTg")
                vg = qkarena.tile([128, NT, 4, 132], BF16, tag="vg")
                nc.vector.memset(vg[:, :, :, 128:129], 1.0)
                for sub in range(3):     # q, k, v strips of this head group
                    wst = wqpool.tile([128, 16, 512], BF16, tag="wst")
                    nc.sync.dma_start(wst[:], wqkv_in[3 * g + sub]
                                      .rearrange("k p f -> p k f"))
                    for t in range(NT):
                        ps = psum_mm.tile([128, 512], F32, tag="ps_qkv")
                        for kk in range(16):
                            nc.tensor.matmul(ps[:], xqT[:, kk, t, :],
                                             wst[:, kk, :],
                                             start=(kk == 0), stop=(kk == 15))
                        dqa_ap = dqa_all[:, t:t + 1]
                        if sub == 2:     # V: dequant into the augmented tile
                            nc.vector.tensor_scalar_mul(
                                vg[:, t, :, 0:128],
                                ps[:].rearrange("p (c f) -> p c f", c=4), dqa_ap)
                        else:
                            cosd = cos_st[:, t, :, :]
                            sind = sin_st[:, t, :, :]
                            rot = rpool.tile([128, 4, 128], BF16, tag="rot")
                            if sub == 0:    # Q: dequant then rope (gpsimd ok)
                                qsc = rpool.tile([128, 512], F32, tag="qsc")
                                nc.vector.tensor_scalar_mul(qsc[:], ps[:], dqa_ap)
                                q3 = qsc[:].rearrange("p (c f) -> p c f", c=4)
                                p1, p2 = q3[:, :, 0:64], q3[:, :, 64:128]
                                t1 = rpool.tile([128, 4, 64], F32, tag="rt1")
                                t2 = rpool.tile([128, 4, 64], F32, tag="rt2")
                                nc.vector.tensor_tensor(t1[:], p1, cosd, OP.mult)
                                nc.gpsimd.tensor_tensor(t2[:], p2, sind, OP.mult)
                                nc.vector.tensor_tensor(rot[:, :, 0:64], t1[:],
                                                        t2[:], OP.subtract)
                                nc.vector.tensor_tensor(t1[:], p2, cosd, OP.mult)
                                nc.gpsimd.tensor_tensor(t2[:], p1, sind, OP.mult)
                                nc.vector.tensor_tensor(rot[:, :, 64:128], t1[:],
                                                        t2[:], OP.add)
                            else:           # K: rope raw ints from PSUM (DVE only;
                                            # dequant folds into the exp scale)
                                k3 = ps[:].rearrange("p (c f) -> p c f", c=4)
                                p1, p2 = k3[:, :, 0:64], k3[:, :, 64:128]
                                t1 = rpool.tile([128, 4, 64], F32, tag="rt1")
                                t2 = rpool.tile([128, 4, 64], F32, tag="rt2")
                                nc.vector.tensor_tensor(t1[:], p1, cosd, OP.mult)
                                nc.vector.tensor_tensor(t2[:], p2, sind, OP.mult)
                                nc.vector.tensor_tensor(rot[:, :, 0:64], t1[:],
                                                        t2[:], OP.subtract)
                                nc.vector.tensor_tensor(t1[:], p2, cosd, OP.mult)
                                nc.vector.tensor_tensor(t2[:], p1, sind, OP.mult)
                                nc.vector.tensor_tensor(rot[:, :, 64:128], t1[:],
                                                        t2[:], OP.add)
                            dst = qTg if sub == 0 else kTg
                            nc.sync.dma_start_transpose(
                                dst[:, :, t, :],
                                rot[:].rearrange("p c f -> p (c f)"))
                # block attention for heads 4g..4g+4
                for blk in range(2):
                    for hh in range(4):
                        expT = [None] * 4
                        for kt in range(4):
                            qn = 512 - kt * 128
                            pss = psum_at.tile([128, 512], F32, tag="ps_sc")
                            nc.tensor.matmul(
                                pss[:, 0:qn],
                                kTg[:, hh, blk * 4 + kt, :],
                                qTg[:, hh, blk * 4:(blk + 1) * 4, :]
                                .rearrange("p c f -> p (c f)")[:, kt * 128:512],
                                start=True, stop=True)
                            ex = apool.tile([128, 512], BF16, tag=f"expT{kt}")
                            nc.scalar.activation(
                                ex[:, 0:qn], pss[:, 0:qn], ACTF.Exp,
                                scale=e_scale[:, blk * 4 + kt:blk * 4 + kt + 1])
                            nc.gpsimd.affine_select(
                                out=ex[:, 0:128], in_=ex[:, 0:128],
                                compare_op=OP.is_ge, fill=0.0,
                                base=0, pattern=[[1, 128]], channel_multiplier=-1)
                            expT[kt] = ex
                        for qt in range(4):
                            psc = psum_cx.tile([128, 132], F32, tag="ps_ctx")
                            for kt in range(qt + 1):
                                nc.tensor.matmul(
                                    psc[:, 0:129],
                                    expT[kt][:, (qt - kt) * 128:(qt - kt) * 128 + 128],
                                    vg[:, blk * 4 + kt, hh, 0:129],
                                    start=(kt == 0), stop=(kt == qt))
                            rl = apool.tile([128, 1], F32, tag="rl")
                            nc.vector.reciprocal(rl[:], psc[:, 128:129])
                            nc.vector.tensor_scalar_mul(
                                ctx_sb[:, blk * 4 + qt, 4 * g + hh, :],
                                psc[:, 0:128], rl[:])
        qkarena.release()
        slotA.release()
        csA.release()

        # o_proj weights prefetch during ctx quant
        wopool = tc.alloc_tile_pool(name="wopool", bufs=1)
        wot = wopool.tile([128, 16, 4, 512], BF16, tag="wot")
        for c4 in range(4):
            nc.sync.dma_start(wot[:, :, c4, :],
                              wo_in[c4].rearrange("k p f -> p k f"))

        # ------------ ctx act-quant + transpose ------------------------------
        ctxqA = tc.alloc_tile_pool(name="ctxqA", bufs=1)
        ctxqT = ctxqA.tile([128, 16, NT, 128], BF16, tag="ctxqT")
        with tc.tile_pool(name="cqpool", bufs=2) as cqpool:
            for t in range(NT):
                src = ctx_sb[:, t, :, :].rearrange("p c f -> p (c f)")
                amax = cqpool.tile([128, 1], F32, tag="camax")
                nc.vector.tensor_reduce(amax[:], src, AX.X, OP.max,
                                        apply_absolute_value=True)
                amc = cqpool.tile([128, 1], F32, tag="camc")
                nc.vector.tensor_scalar_max(amc[:], amax[:], 1e-5)
                rec = cqpool.tile([128, 1], F32, tag="crec")
                nc.vector.reciprocal(rec[:], amc[:])
                s_c = cqpool.tile([128, 1], F32, tag="cs")
                nc.vector.tensor_scalar_mul(s_c[:], rec[:], 127.0)
                nc.vector.tensor_scalar(dqc_all[:, t:t + 1], amc[:], dq_b[:, 1:2],
                                        None, OP.mult)
                mg = cqpool.tile([128, H], F32, tag="cmg")
                nc.scalar.activation(mg[:], src, ACTF.Identity,
                                     bias=magic_ap, scale=s_c[:])
                cq = cqpool.tile([128, H], BF16, tag="cq")
                nc.vector.tensor_scalar_sub(cq[:], mg[:], float(MAGIC))
                nc.sync.dma_start_transpose(ctxqT[:, :, t, :], cq[:])
        ctxA.release()

        # ------------ o_proj + residual + ffn rmsnorm (fused per tile) -------
        hnA = tc.alloc_tile_pool(name="hnA", bufs=1, side="right")
        hnT = hnA.tile([128, 16, NT, 128], F16, tag="hnT")
        with tc.tile_pool(name="opool", bufs=3) as opool, \
             tc.tile_pool(name="o_psum", bufs=3, space="PSUM") as psum_o:
            for t in range(NT):
                x1t = opool.tile([128, H], F32, tag="x1t")
                for nn in range(4):
                    ps = psum_o.tile([128, 512], F32, tag="ps_o")
                    for kk in range(16):
                        nc.tensor.matmul(ps[:], ctxqT[:, kk, t, :],
                                         wot[:, kk, nn, :],
                                         start=(kk == 0), stop=(kk == 15))
                    xs = opool.tile([128, 512], F32, tag="xs")
                    nc.scalar.dma_start(xs[:], x_in[t * 128:(t + 1) * 128,
                                                    nn * 512:(nn + 1) * 512])
                    tmp = opool.tile([128, 512], F32, tag="o_tmp")
                    nc.vector.tensor_scalar_mul(tmp[:], ps[:], dqc_all[:, t:t + 1])
                    nc.gpsimd.tensor_tensor(x1t[:, nn * 512:(nn + 1) * 512],
                                            tmp[:], xs[:], OP.add)
                nc.sync.dma_start(x1_d[t, :, :], x1t[:])
                # ffn rmsnorm on the in-SBUF x1 tile
                ssq = opool.tile([128, 1], F32, tag="ssq2")
                junk = opool.tile([128, H], BF16, tag="njunk2")
                nc.scalar.activation(junk[:], x1t[:], ACTF.Square, accum_out=ssq[:])
                msq = opool.tile([128, 1], F32, tag="msq2")
                nc.vector.tensor_scalar(msq[:], ssq[:], 1.0 / H, EPS, OP.mult, OP.add)
                sd = opool.tile([128, 1], F32, tag="sd2")
                nc.scalar.activation(sd[:], msq[:], ACTF.Sqrt)
                rstd = opool.tile([128, 1], F32, tag="rstd2")
                nc.vector.reciprocal(rstd[:], sd[:])
                hn16 = opool.tile([128, H], F16, tag="hn16")
                if skip_fnw:
                    nc.scalar.activation(hn16[:], x1t[:], ACTF.Identity,
                                         scale=rstd[:])
                else:
                    hn = opool.tile([128, H], F32, tag="hnf")
                    nc.vector.tensor_scalar_mul(hn[:], x1t[:], rstd[:])
                    nc.vector.tensor_tensor(hn16[:], hn[:], fnw_b[:], OP.mult)
                nc.sync.dma_start_transpose(hnT[:, :, t, :], hn16[:])
        ctxqA.release()
        wopool.release()

        # actqT arena + ffn_down weight prefetch start before ffn_up runs
        aqA = tc.alloc_tile_pool(name="aqA", bufs=1)
        actqT = aqA.tile([128, 32, NT, 128], BF16, tag="actqT")
        dpool = tc.alloc_tile_pool(name="dpool", bufs=3)
        wdt_tiles = {}

        def load_dn_strip(nn):
            ta = dpool.tile([128, 16, 512], BF16, tag="wdtH")
            nc.sync.dma_start(ta[:], wdn_in[nn, 0:16].rearrange("k p f -> p k f"))
            tb = dpool.tile([128, 16, 512], BF16, tag="wdtH")
            nc.sync.dma_start(tb[:], wdn_in[nn, 16:32].rearrange("k p f -> p k f"))
            wdt_tiles[nn] = (ta, tb)

        load_dn_strip(0)

        # ------------ ffn up (fp16, unpaired gate/val) -> act_d --------------
        svpool = tc.alloc_tile_pool(name="svpool", bufs=1, side="right")
        sg_t = []
        for t in range(NT):
            sgt = svpool.tile([128, 512], BF16, tag=f"sg{t}")
            sg_t.append(sgt)
        with tc.tile_pool(name="upool", bufs=2, side="right") as upool, \
             tc.tile_pool(name="fpool", bufs=3) as fpool, \
             tc.tile_pool(name="up_psum", bufs=3, space="PSUM") as psum_up:
            for j in range(16):          # gate0, val0, gate1, val1, ...
                wu = upool.tile([128, 16, 512], F16, tag="wup")
                nc.sync.dma_start(wu[:], wup_in[j].rearrange("k p f -> p k f"))
                for t in range(NT):
                    ps = psum_up.tile([128, 512], F32, tag="ps_up")
                    for kk in range(16):
                        nc.tensor.matmul(ps[:], hnT[:, kk, t, :], wu[:, kk, :],
                                         start=(kk == 0), stop=(kk == 15))
                    if j % 2 == 0:       # gate strip: stage silu(gate)
                        sgm = fpool.tile([128, 512], F32, tag="sgm")
                        nc.scalar.activation(sgm[:], ps[:], ACTF.Sigmoid)
                        nc.vector.tensor_tensor(sg_t[t][:], sgm[:], ps[:], OP.mult)
                    else:                # val strip: act = silu(gate)*val
                        av = fpool.tile([128, 512], F16, tag="av")
                        nc.vector.tensor_tensor(av[:], sg_t[t][:], ps[:], OP.mult)
                        i = j // 2
                        nc.scalar.dma_start(act_d[t, :, i * 512:(i + 1) * 512],
                                            av[:])
        svpool.release()
        hnA.release()

        # ------------ act quant + transpose (overlaps ffn_up tail) -----------
        with tc.tile_pool(name="aqpool", bufs=2) as aqpool:
            for t in range(NT):
                at = aqpool.tile([128, INTER], F16, tag="at")
                nc.sync.dma_start(at[:], act_d[t, :, :])
                amax = aqpool.tile([128, 1], F32, tag="aamax")
                nc.vector.tensor_reduce(amax[:], at[:], AX.X, OP.max,
                                        apply_absolute_value=True)
                amc = aqpool.tile([128, 1], F32, tag="aamc")
                nc.vector.tensor_scalar_max(amc[:], amax[:], 1e-5)
                rec = aqpool.tile([128, 1], F32, tag="arec")
                nc.vector.reciprocal(rec[:], amc[:])
                s_a = aqpool.tile([128, 1], F32, tag="as")
                nc.vector.tensor_scalar_mul(s_a[:], rec[:], 127.0)
                nc.vector.tensor_scalar(dqact_all[:, t:t + 1], amc[:], dq_b[:, 2:3],
                                        None, OP.mult)
                for half in range(2):
                    sl = slice(half * 2048, (half + 1) * 2048)
                    mg = aqpool.tile([128, 2048], F32, tag="amg")
                    nc.scalar.activation(mg[:], at[:, sl], ACTF.Identity,
                                         bias=magic_ap, scale=s_a[:])
                    aq = aqpool.tile([128, 2048], BF16, tag="aq")
                    nc.vector.tensor_scalar_sub(aq[:], mg[:], float(MAGIC))
                    nc.sync.dma_start_transpose(
                        actqT[:, half * 16:(half + 1) * 16, t, :], aq[:])

        # ------------ ffn down + residual -> out -----------------------------
        with tc.tile_pool(name="dopool", bufs=3) as dopool, \
             tc.tile_pool(name="dn_psum", bufs=3, space="PSUM") as psum_dn:
            for nn in range(4):
                if nn not in wdt_tiles:
                    load_dn_strip(nn)
                ta, tb = wdt_tiles[nn]
                for t in range(NT):
                    ps = psum_dn.tile([128, 512], F32, tag="ps_dn")
                    for kk in range(32):
                        w_ap = ta[:, kk, :] if kk < 16 else tb[:, kk - 16, :]
                        nc.tensor.matmul(ps[:], actqT[:, kk, t, :], w_ap,
                                         start=(kk == 0), stop=(kk == 31))
                    x1s = dopool.tile([128, 512], F32, tag="x1s")
                    nc.scalar.dma_start(x1s[:], x1_d[t, :, nn * 512:(nn + 1) * 512])
                    tmp = dopool.tile([128, 512], F32, tag="d_tmp")
                    nc.vector.tensor_scalar_mul(tmp[:], ps[:], dqact_all[:, t:t + 1])
                    ot = dopool.tile([128, 512], F32, tag="ot")
                    nc.gpsimd.tensor_tensor(ot[:], tmp[:], x1s[:], OP.add)
                    nc.sync.dma_start(out_d[t * 128:(t + 1) * 128,
                                            nn * 512:(nn + 1) * 512], ot[:])
        dpool.release()
        aqA.release()
        nwpool.release()
        perm.release()

    nc.compile()
    return nc


_NC_CACHE = {}


def _get_nc(skip_anw, skip_fnw):
    key = (skip_anw, skip_fnw)
    if key not in _NC_CACHE:
        _NC_CACHE[key] = build_program(skip_anw, skip_fnw)
    return _NC_CACHE[key]


def _ternarize(w):
    """Exact reference _weight_quant: per-tensor ternary + fp32 scale."""
    w = np.asarray(w, np.float32)
    m = np.float32(max(np.float32(np.mean(np.abs(w))), np.float32(1e-5)))
    scale = np.float32(1.0) / m
    t = np.clip(np.rint(w * scale), -1, 1)
    return t, np.float32(m)


def _host_inputs(x, attn_norm_w, ffn_norm_w, qkv_w, o_w, ffn_up_w, ffn_down_w):
    x = np.ascontiguousarray(np.asarray(x, np.float32))
    anw = np.ascontiguousarray(np.asarray(attn_norm_w, np.float32))
    fnw = np.ascontiguousarray(np.asarray(ffn_norm_w, np.float32))

    tq, m_qkv = _ternarize(np.asarray(qkv_w, np.float32).T)       # [2048, 6144]
    to, m_o = _ternarize(np.asarray(o_w, np.float32).T)           # [2048, 2048]
    td, m_dn = _ternarize(np.asarray(ffn_down_w, np.float32).T)   # [4096, 2048]

    wqkv3 = np.ascontiguousarray(
        tq.reshape(16, 128, 12, 512).transpose(2, 0, 1, 3)[QKV_ORDER]
    ).astype(ml_dtypes.bfloat16)
    wo3 = np.ascontiguousarray(
        to.reshape(16, 128, 4, 512).transpose(2, 0, 1, 3)
    ).astype(ml_dtypes.bfloat16)
    wdn3 = np.ascontiguousarray(
        td.reshape(32, 128, 4, 512).transpose(2, 0, 1, 3)
    ).astype(ml_dtypes.bfloat16)
    wup = np.asarray(ffn_up_w, np.float32).T                      # [2048, 8192]
    wup_h = np.ascontiguousarray(
        wup.reshape(16, 128, 16, 512).transpose(2, 0, 1, 3)[UP_ORDER]
    ).astype(np.float16)

    dqs = np.array([m_qkv / 127.0, m_o / 127.0, m_dn / 127.0, 0.0], np.float32)

    inv = 1.0 / (THETA ** (np.arange(0, HD, 2, dtype=np.float32) / HD))
    tpos = np.arange(S, dtype=np.float32)
    fr = np.outer(tpos, inv)                     # [S, 64]
    cosf = np.cos(fr).astype(np.float32)
    sinf = np.sin(fr).astype(np.float32)

    in_maps = []
    for c in range(NCORES):
        b = c // 4
        t0 = (c % 4) * R
        in_maps.append({
            "x_sh": np.ascontiguousarray(x[b, t0:t0 + R, :]),
            "cosb": np.ascontiguousarray(cosf[t0:t0 + R].reshape(NT, 128, 64)),
            "sinb": np.ascontiguousarray(sinf[t0:t0 + R].reshape(NT, 128, 64)),
            "attn_norm_w": anw, "ffn_norm_w": fnw, "dqs": dqs,
            "wqkv3": wqkv3, "wo3": wo3, "wup_h": wup_h, "wdn3": wdn3,
        })
    skip_anw = bool(np.all(anw == 1.0))
    skip_fnw = bool(np.all(fnw == 1.0))
    return in_maps, skip_anw, skip_fnw


def run(trace=False, **inputs):
    in_maps, skip_anw, skip_fnw = _host_inputs(**inputs)
    nc = _get_nc(skip_anw, skip_fnw)
    res = run_bass_kernel_spmd(nc, in_maps, list(range(NCORES)), trace=trace)
    out = np.empty((B, S, H), np.float32)
    for c in range(NCORES):
        b = c // 4
        t0 = (c % 4) * R
        out[b, t0:t0 + R, :] = res.results[c]["out_sh"]
    return out, res


def kernel(**inputs):
    out, _ = run(trace=False, **inputs)
    return out
